# revision 24
# baseline (speedup 1.0000x reference)
"""DTSemNet forward (nn_DTSemNet_54528904790526) on 8 TRN2 NeuronCores.

Math: the reference computes
    x = in_x @ W1.T + b1                       [B, 2047]
    h = [relu(x), relu(-x)]                    [B, 4094]
    z = h @ L.T                                [B, 2048]   (frozen 0/1 leaf routing)
    out[b, a] = max over leaves ell with (ell % 10 == a) of z[b, ell]

L is the complete-binary-tree path matrix: for leaf ell the row is 1 everywhere
except, at each of the 11 path nodes, the half corresponding to the direction
NOT taken. Hence
    z[b, ell] = sum_i |x_i|  -  sum_{path nodes} penalty,
    penalty   = relu(-x_node) when going left, relu(x_node) when going right.
So z = S_abs - cost(leaf) where cost is an 11-level tree DP — this replaces the
dense [B,4094]x[4094,2048] matmul (80% of reference FLOPs) with O(n_leaves)
vector work.

Per-core shard: batch rows (data parallel over 8 cores, 2048 rows each).
Matmul runs in fp8-e4m3 with MatmulPerfMode.DoubleRow: two 128-deep k-tiles
are packed per instruction (stationary x-block [128, 2, 128], moving weights
[128, 2, 512]), doubling PE throughput vs fp32r/bf16. Inputs are quantized
host-side with power-of-2 scales (in_x x32, W1 x4096 - both fit e4m3's +-240
range; max |in_x| ~ 5.8 -> 186) and the 2^-17 descale is folded into the
activation scale. Quantization noise: ~2.5% relative per product term, so
std(dx) ~ 0.015 per node preactivation; S_abs sums 2047 |x| values ->
error std ~ 0.65 absolute on a ~940 output => ~3e-3 max relative error
(tolerance 2e-2). Bias is a K=1 fp8 matmul (ones=16, b1 x 8192).
The DP runs on the vector engine in bf16 (cost values are O(10)
sums of relu's; bf16 absolute error ~3e-3 => ~3e-6 relative on the output).

Leaf costs are kept in a "split" layout (evens | odds by natural leaf index)
so every DP write is contiguous; the parent interleave is a strided read and
each level is ONE tensor_tensor (parent broadcast via a stride-0 AP dim).
Group-min: leaf ell = 2m+s, ell % 10 = a  <=>  s = a%2, m ≡ a//2 (mod 5);
a TT-min fold 1020->510 per half (multiples of 5 preserve residues) then one
[r=5, j=102]-structured strided reduce per half + a 4-element leftover fixup.

Measured (steady state, per core sweep of 2048 rows): ~225-255 us, vs a
~190 us pure-matmul floor (PE at 2.8 GHz, fp32r ~280 ns per 128x128x512 MM)
— the dense reference (two matmuls incl. [B,4094]x[4094,2048]) would be
~5x slower on the same PE.
"""
import sys

sys.path.insert(0, "/opt/trn_rl_repo")
from contextlib import ExitStack

import numpy as np

import concourse.bass as bass
import concourse.tile as tile
from concourse import bacc, mybir
from concourse.bass_utils import run_bass_kernel_spmd

# problem shape (hardcoded per contract)
B = 16384
D = 2048
N = 2047          # internal nodes
NP = 2048         # N padded to even/512-multiple (fp32r needs even free dims)
HEIGHT = 11
NL = 2048         # leaves
OUT = 10
NCORES = 8
BC = B // NCORES  # batch rows per core (2048)
KT = D // 128     # 16 k-tiles
BT = BC // 128    # 16 batch tiles per core
CHUNKS = [(0, 512), (512, 1024), (1024, 1536), (1536, 2048)]

f32 = mybir.dt.float32
f32r = mybir.dt.float32r
f8 = mybir.dt.float8e4
bf16 = mybir.dt.bfloat16
DR = mybir.MatmulPerfMode.DoubleRow

# fp8 quantization scales (powers of 2: exactly representable, descale is a
# single activation-scale multiply)
XS = 32.0          # in_x scale: |x| <= ~5.8 -> 186 < 240 (e4m3 max)
WS = 4096.0        # W1 scale: |W1| <= 0.0221 -> 90.5
BS = 8192.0        # b1 scale: |b1| <= 0.0221 -> 181
ONESV = XS * WS / BS   # 16.0: ones-row value so bias descales identically
INV = 1.0 / (XS * WS)  # 2^-17 PSUM descale
KP = KT // 2       # 8 k-tile pairs (DoubleRow processes 2 k-tiles/instr)
ADD = mybir.AluOpType.add
MIN = mybir.AluOpType.min
SUB = mybir.AluOpType.subtract
MULT = mybir.AluOpType.mult
RELU = mybir.ActivationFunctionType.Relu
MAX = mybir.AluOpType.max
AXX = mybir.AxisListType.X


def build_kernel(bt=BT, reps=1, loop_reps=None, mode="full"):
    """bt: number of batch tiles (128 rows each) this kernel processes.
    reps: python-unrolled repeats of the whole per-tile pipeline.
    loop_reps: device-side For_i repeats (for timing probes).
    mode: "full" | "nodp" (skip tree DP/mins) | "mmonly" (matmuls only)."""
    nc = bacc.Bacc("TRN2")
    # in_x.T shard (fp8, pre-scaled by XS), pre-blocked host-side as
    # [bt][128 p][KT k][128 m] with p = contraction row % 128, so each SBUF
    # partition reads one contiguous 2KB run per batch tile.
    xt = nc.dram_tensor("xt", [bt * 128, KT * 128], f8, kind="ExternalInput")
    # W1.T (x WS, fp8, padded to NP cols) in DoubleRow pair layout:
    # wp[j*128+p, i*NP+m] = W1.T[(2j+i)*128+p, m] * WS
    wp = nc.dram_tensor("wp", [KP * 128, 2 * NP], f8, kind="ExternalInput")
    ones = nc.dram_tensor("ones", [1, 128], f8, kind="ExternalInput")
    wtb4 = nc.dram_tensor("wtb4", [128, NP], f8, kind="ExternalInput")
    out = nc.dram_tensor("out", [bt * 128, OUT], f32, kind="ExternalOutput")

    with tile.TileContext(nc) as tc, ExitStack() as ctx:
        wt_pool = ctx.enter_context(tc.tile_pool(name="wt", bufs=1))
        xt_pool = ctx.enter_context(tc.tile_pool(name="xt", bufs=3))
        ps_pool = ctx.enter_context(tc.tile_pool(name="ps", bufs=2, space="PSUM"))
        pen_pool = ctx.enter_context(tc.tile_pool(name="pen", bufs=2))
        dp_pool = ctx.enter_context(tc.tile_pool(name="dp", bufs=3))
        sm_pool = ctx.enter_context(tc.tile_pool(name="sm", bufs=4))

        # resident weights: 8 k-tile pairs + bias row
        wps = []
        for j in range(KP):
            wj = wt_pool.tile([128, 2 * NP], f8, tag=f"wp{j}")
            nc.sync.dma_start(wj[:], wp[j * 128:(j + 1) * 128, :])
            wps.append(wj)
        # b1 and ones rows replicated at partitions 0/32/64/96 so the four
        # K=1 bias matmuls can run as concurrent PE row-group tiles
        wtb_t = wt_pool.tile([128, NP], f8, tag="wtb")
        nc.sync.dma_start(wtb_t[:], wtb4[:, :])
        ones_t = wt_pool.tile([128, 128], f8, tag="ones")
        for rg in range(4):
            nc.sync.dma_start(ones_t[32 * rg:32 * rg + 1, :], ones[0:1, :])

        def body():
            # tiles processed in PAIRS: matmul+ACT per tile, but ONE DVE
            # chain per pair (a q=2 dim on every DP/fold AP) — halves the
            # per-instruction fixed costs on the bottleneck engine (DVE).
            for t in range(0, bt, 2):
                # pen for the pair, interleaved penI per q-slot:
                # pen2t[q*2NP + 2n + u]: u=0 pl[n], u=1 pr[n]
                pen2t = pen_pool.tile([128, 2 * 2 * NP], bf16, tag="pen")
                sacc = sm_pool.tile([128, 4], f32, tag="sacc")

                for q in range(2):
                    tt = t + q
                    c_lo = tt * 128
                    c_hi = (tt + 1) * 128
                    xt_t = xt_pool.tile([128, KT * 128], f8, tag="xt")
                    nc.sync.dma_start(xt_t[:], xt[c_lo:c_hi, :])

                    if mode == "dponly":
                        nc.gpsimd.memset(pen2t[:, q * 2 * NP:(q + 1) * 2 * NP],
                                         0.5)
                        nc.gpsimd.memset(sacc[:, 2 * q:2 * q + 2], 1.0)
                        continue
                    # single 4-bank PSUM tile: matmuls target bank-aligned
                    # 512-slices, activations read the full 2048 in one instr
                    pst = ps_pool.tile([128, NP], f32, tag="ps", name="ps")
                    # pair-outer order: 4 consecutive DoubleRow matmuls share
                    # the stationary x pair-block [128, 2, 128]
                    for j in range(KP):
                        lhsT = xt_t[:, j * 256:(j + 1) * 256].rearrange(
                            "p (two m) -> p two m", two=2)
                        wv = wps[j][:].rearrange("p (two m) -> p two m", two=2)
                        for ci, (c0, c1) in enumerate(CHUNKS):
                            nc.tensor.matmul(
                                pst[:, c0:c1],
                                lhsT,
                                wv[:, :, c0:c1],
                                start=(j == 0), stop=False,
                                perf_mode=DR,
                            )
                    for ci, (c0, c1) in enumerate(CHUNKS):
                        bp = 32 * ci
                        nc.tensor.matmul(
                            pst[:, c0:c1],
                            ones_t[bp:bp + 1, 0:128],
                            wtb_t[bp:bp + 1, c0:c1],
                            start=False, stop=True,
                            tile_position=(bp, 0),
                        )
                    if mode == "mmonly":
                        outsb = sm_pool.tile([128, OUT], f32, tag="outmm")
                        nc.scalar.copy(outsb[:], pst[:, 0:OUT])
                        nc.gpsimd.dma_start(out[c_lo:c_hi, :], outsb[:])
                        continue
                    # full-width activations on ACT (GPSIMD can't read PSUM),
                    # written INTERLEAVED: pen[2n] = pl[n] = relu(-x_n*INV),
                    # pen[2n+1] = pr[n] = relu(x_n*INV). This makes every DP
                    # read/write unit-stride in natural child order. Row sums
                    # accumulate for S_abs.
                    penV = pen2t[:, q * 2 * NP:(q + 1) * 2 * NP].rearrange(
                        "p (n u) -> p u n", u=2)
                    nc.scalar.activation(
                        penV[:, 0, :], pst[:], RELU, scale=-INV,
                        accum_out=sacc[:, 2 * q + 1:2 * q + 2],
                    )
                    nc.scalar.activation(
                        penV[:, 1, :], pst[:], RELU, scale=INV,
                        accum_out=sacc[:, 2 * q:2 * q + 1],
                    )
                if mode == "mmonly":
                    continue

                sabs = sm_pool.tile([128, 2], f32, tag="sabs")
                nc.vector.tensor_reduce(
                    sabs[:].rearrange("p (q x) -> p q x", x=1),
                    sacc[:].rearrange("p (q s) -> p q s", q=2),
                    axis=AXX, op=ADD)
                if mode == "nodp":
                    outsb = sm_pool.tile([128, 2 * OUT], f32, tag="outsb")
                    nc.scalar.copy(outsb[:, 0:4], sacc[:])
                    nc.gpsimd.dma_start(
                        out[t * 128:(t + 1) * 128, 0:4], outsb[:, 0:4])
                    continue

                # ---- tree DP, natural child order, both tiles per instr ----
                # One TT per level d=1..10 builds c_{d+1}: out[q, 2j+u] =
                # par[q, j] + penI[q, 2*(n0+j)+u]. All APs unit-stride
                # innermost (par broadcast via stride-0).
                penQ = pen2t[:].rearrange("p (q j u) -> p q j u", q=2, u=2)
                par = pen2t[:].rearrange("p (q c) -> p q c", q=2)[:, :, 0:2]
                c11 = None
                for d in range(1, HEIGHT):
                    w = 1 << d          # number of level-d nodes = parents
                    n0 = w - 1          # first node index of level d
                    nxt = dp_pool.tile([128, 2 * 2 * w], bf16,
                                       tag=f"lvl{d + 1}")
                    nxtQ = nxt[:].rearrange("p (q c) -> p q c", q=2)
                    out4 = nxt[:].rearrange("p (q j u) -> p q j u", q=2, u=2)
                    pen4 = penQ[:, :, n0:n0 + w, :]
                    par4 = par.rearrange("p q (j x) -> p q j x", x=1)
                    par4 = par4.broadcast_to([128, 2, w, 2])
                    nc.vector.tensor_tensor(out4, par4, pen4, op=ADD)
                    par = nxtQ
                    c11 = nxt

                # par/c11 hold leaf costs [p, q, 2048] (leaf ell = a mod 10).
                # group-min: unit-stride TT-min folds by multiples of 10
                # (2040->1020->510), one strided reduce [r=10, j=51], then an
                # 8-elem tail fixup (leaves 2040:2047).
                c11Q = par
                tmp = sm_pool.tile([128, 2 * OUT], bf16, tag="mins")
                tmpQ = tmp[:].rearrange("p (q a) -> p q a", q=2)
                fold = dp_pool.tile([128, 2 * 1024], bf16, tag="fold")
                foldQ = fold[:].rearrange("p (q c) -> p q c", q=2)
                nc.vector.tensor_tensor(
                    foldQ[:, :, 0:1020], c11Q[:, :, 0:1020],
                    c11Q[:, :, 1020:2040], op=MIN,
                )
                nc.vector.tensor_tensor(
                    foldQ[:, :, 0:510], foldQ[:, :, 0:510],
                    foldQ[:, :, 510:1020], op=MIN,
                )
                src = foldQ[:, :, 0:510].rearrange(
                    "p q (j r) -> p q r j", r=10)
                nc.vector.tensor_reduce(tmpQ, src, axis=AXX, op=MIN)
                nc.vector.tensor_tensor(
                    tmpQ[:, :, 0:8], tmpQ[:, :, 0:8],
                    c11Q[:, :, 2040:2048], op=MIN,
                )

                outsb = sm_pool.tile([128, 2 * OUT], f32, tag="outsb")
                outQ = outsb[:].rearrange("p (q a) -> p q a", q=2)
                # out[:, q, a] = S_abs[q] - min_cost[q, a]
                sabs_b = sabs[:].rearrange("p (q x) -> p q x", x=1)
                sabs_b = sabs_b.broadcast_to([128, 2, OUT])
                nc.vector.tensor_tensor(outQ, sabs_b, tmpQ, op=SUB)
                nc.gpsimd.dma_start(
                    out[t * 128:(t + 2) * 128, :].rearrange(
                        "(q m) a -> m q a", q=2),
                    outQ)

        if loop_reps is not None:
            with tc.For_i(0, loop_reps):
                body()
        else:
            for _ in range(reps):
                body()

    nc.finalize()
    return nc


_NC_CACHE = {}


def _get_nc():
    key = (BT, 1)
    if key not in _NC_CACHE:
        _NC_CACHE[key] = build_kernel()
    return _NC_CACHE[key]


def _to_f8(a):
    import ml_dtypes
    return np.clip(a, -240.0, 240.0).astype(ml_dtypes.float8_e4m3)


def marshal_xt(in_x_shard):
    """[BC, D] rows -> [BT*128, KT*128] blocked so that SBUF partition p of
    batch-tile t reads one contiguous 2KB run: out[t*128+p, k*128+m] =
    in_x_shard[t*128+m, k*128+p]. Scaled by XS and quantized to fp8."""
    a = in_x_shard.reshape(BT, 128, KT, 128)        # [t, m, k, p]
    a = np.ascontiguousarray(
        a.transpose(0, 3, 2, 1).reshape(BT * 128, KT * 128))
    return _to_f8(a * XS)


def make_in_maps(inputs):
    in_x = np.asarray(inputs["in_x"], np.float32)
    W1 = np.asarray(inputs["W1"], np.float32)
    b1 = np.asarray(inputs["b1"], np.float32)
    # host-side layout marshaling: blocked-transposed fp8 activations,
    # transposed fp8 weights in DoubleRow pair layout, fp8 bias row
    wt = np.zeros((D, NP), np.float32)
    wt[:, :N] = W1.T
    wp = _to_f8(
        (wt * WS).reshape(KP, 2, 128, NP).transpose(0, 2, 1, 3)
        .reshape(KP * 128, 2 * NP))
    b_row = np.zeros((NP,), np.float32)
    b_row[:N] = b1
    wtb4 = _to_f8(np.broadcast_to(b_row * BS, (128, NP)))
    ones = np.full((1, 128), ONESV)
    return [
        {"xt": marshal_xt(in_x[c * BC:(c + 1) * BC]), "wp": wp,
         "ones": _to_f8(ones), "wtb4": wtb4}
        for c in range(NCORES)
    ]


def kernel(in_x, W1, b1, L, A):
    in_maps = make_in_maps({"in_x": in_x, "W1": W1, "b1": b1})
    nc = _get_nc()
    res = run_bass_kernel_spmd(nc, in_maps, core_ids=list(range(NCORES)))
    return np.concatenate([res.results[c]["out"] for c in range(NCORES)], axis=0)

